# revision 12
# baseline (speedup 1.0000x reference)
"""Distributed Trainium2 (8 NeuronCores) kernel for nn_BiGraphContrastLayer.

Strategy
--------
All graph edges go src-type -> predict-type, so for the 32768 source nodes the
GraphConv reduces to prelu(m + b) with m = feat @ W (self-loop only), and both
graphs share it.  Per predict node j:

    h_g[j]   = prelu((m[dst_j] + sum_{edges g into j} m[src]) / (1+deg_g(j)) + b)
    pos[j]   = <hp[j], hn[j]>                  (normalized rows)
    neg1[j]  = <hp[j], sum_neg nsrc[src]> + pos[j]
    neg2[j]  = <hn[j], sum_pos nsrc[src]> + pos[j]
    loss     = log(sum exp(pos|neg1|neg2)) - sum pos

with nsrc = normalize(prelu(m_src + b)) shared by both views.

Device plan (SPMD x8): predict nodes are greedy-balanced into 256 bins of 128
(32 bins per core); edges are routed to the bin owning their dst.  Each core
builds its 4096-row slice of a bf16 gather table T = [m_src | nsrc] (512 wide),
an AllGather replicates T, then per bin one dma_gather fetches all edge rows
and per-128-edge chunks a one-hot matmul (iota==slot) segment-sums them into a
[128, 512] PSUM bank, seeded by m_dst via a [W | 0] matmul.  Epilogue computes
h, normalized rows, the three dot columns and exp-partials.  Host reassembles
rows and finishes the scalar log-sum-exp reduction.
"""
from contextlib import ExitStack
import heapq

import numpy as np
import ml_dtypes

import concourse.bass as bass
import concourse.bacc as bacc
import concourse.mybir as mybir
import concourse.tile as tile
from concourse.bass_utils import run_bass_kernel_spmd

bf16 = ml_dtypes.bfloat16
F32 = mybir.dt.float32
BF16 = mybir.dt.bfloat16
I16 = mybir.dt.int16
AF = mybir.ActivationFunctionType
ALU = mybir.AluOpType

N_SRC = 32768
N_DST = 32768
D = 256
D2 = 2 * D
C = 8            # cores
B = 32           # dst bins per core
S = 128          # dst nodes per bin
NBINS = C * B
SRC_PER_CORE = N_SRC // C      # 4096
EPS = 1e-8

LAST_RESULTS = None            # test.py introspection


# ----------------------------------------------------------------- host side

def _route(src, dstj, node_bin, node_slot):
    """Group edges by dst bin; pad each bin to K*128 with dead (idx 0, off 128)."""
    b_arr = node_bin[dstj]
    o = np.argsort(b_arr, kind="stable")
    b_sorted, s_sorted, off_sorted = b_arr[o], src[o], node_slot[dstj][o]
    counts = np.bincount(b_sorted, minlength=NBINS)
    K = int(np.ceil(counts.max() / 128))
    starts = np.concatenate([[0], np.cumsum(counts)])
    idx = np.zeros((NBINS, K * 128), dtype=np.int16)
    off = np.full((NBINS, K * 128), 128, dtype=np.float32)
    for b in range(NBINS):
        c = counts[b]
        idx[b, :c] = s_sorted[starts[b]:starts[b] + c]
        off[b, :c] = off_sorted[starts[b]:starts[b] + c]
    return idx, off, K


def _preprocess(src, dst, neg_src, neg_dst):
    dstj = (dst - N_SRC).astype(np.int64)
    ndstj = (neg_dst - N_SRC).astype(np.int64)
    deg_p = np.bincount(dstj, minlength=N_DST)
    deg_n = np.bincount(ndstj, minlength=N_DST)
    tot = deg_p + deg_n

    order = np.argsort(-tot, kind="stable")
    bin_cnt = np.zeros(NBINS, dtype=np.int32)
    node_bin = np.empty(N_DST, dtype=np.int64)
    node_slot = np.empty(N_DST, dtype=np.int64)
    heap = [(0, b) for b in range(NBINS)]
    heapq.heapify(heap)
    for j in order:
        while True:
            t, b = heapq.heappop(heap)
            if bin_cnt[b] < S:
                break
        node_bin[j] = b
        node_slot[j] = bin_cnt[b]
        bin_cnt[b] += 1
        if bin_cnt[b] < S:
            heapq.heappush(heap, (t + tot[j], b))

    nodes_at = np.empty((NBINS, S), dtype=np.int64)
    nodes_at[node_bin, node_slot] = np.arange(N_DST)

    idx_p, off_p, K_p = _route(src.astype(np.int64), dstj, node_bin, node_slot)
    idx_n, off_n, K_n = _route(neg_src.astype(np.int64), ndstj, node_bin, node_slot)

    invd_p = (1.0 / (1.0 + deg_p[nodes_at])).astype(np.float32)   # [NBINS, S]
    invd_n = (1.0 / (1.0 + deg_n[nodes_at])).astype(np.float32)
    return dict(nodes_at=nodes_at, idx_p=idx_p, off_p=off_p, K_p=K_p,
                idx_n=idx_n, off_n=off_n, K_n=K_n,
                invd_p=invd_p, invd_n=invd_n)


def _wrap_idx(idx):
    """[NBINS, K*128] -> per-core [128, B, K*8] int16 dma_gather layout:
    element i -> partition i%16 (replicated x8), column i//16."""
    K8 = idx.shape[1] // 16
    a = idx.reshape(NBINS, K8, 16).transpose(0, 2, 1)        # [NBINS, 16, K8]
    a = np.tile(a, (1, 8, 1))                                # [NBINS, 128, K8]
    a = a.reshape(C, B, 128, K8).transpose(0, 2, 1, 3)       # [C, 128, B, K8]
    return np.ascontiguousarray(a.astype(np.int16))


def _layout_off(off, K):
    """[NBINS, K*128] -> per-core [128, B*K] with col b*K+k = off[bin, k*128+e]."""
    a = off.reshape(NBINS, K, 128).transpose(2, 0, 1)        # [128, NBINS, K]
    a = a.reshape(128, C, B * K).transpose(1, 0, 2)          # [C, 128, B*K]
    return np.ascontiguousarray(a.astype(np.float32))


# --------------------------------------------------------------- device side

def _build(K_p, K_n, n_bins=B, do_ag=True, do_main=True, do_tableA=True, do_stageC=True, d_level=5):
    nc = bacc.Bacc("TRN2", target_bir_lowering=False, debug=False, num_devices=C)

    feat_src_d = nc.dram_tensor("feat_src", [SRC_PER_CORE, D], F32, kind="ExternalInput")
    feat_dst_d = nc.dram_tensor("feat_dst", [B, S, D], F32, kind="ExternalInput")
    w_d = nc.dram_tensor("w", [D, D], F32, kind="ExternalInput")
    bvec_d = nc.dram_tensor("bvec", [1, D], F32, kind="ExternalInput")
    a_d = nc.dram_tensor("a", [1, 1], F32, kind="ExternalInput")
    idxp_d = nc.dram_tensor("idx_p", [128, B, K_p * 8], I16, kind="ExternalInput")
    idxn_d = nc.dram_tensor("idx_n", [128, B, K_n * 8], I16, kind="ExternalInput")
    offp_d = nc.dram_tensor("off_p", [128, B * K_p], F32, kind="ExternalInput")
    offn_d = nc.dram_tensor("off_n", [128, B * K_n], F32, kind="ExternalInput")
    invp_d = nc.dram_tensor("invd_p", [128, B], F32, kind="ExternalInput")
    invn_d = nc.dram_tensor("invd_n", [128, B], F32, kind="ExternalInput")

    outh_d = nc.dram_tensor("out_h", [B, S, D], F32, kind="ExternalOutput")
    outp_d = nc.dram_tensor("out_partials", [128, 2], F32, kind="ExternalOutput")

    tbl_local = nc.dram_tensor("tbl_local", [SRC_PER_CORE, D2], BF16, kind="Internal")
    tbl_full = nc.dram_tensor("tbl_full", [N_SRC, D2], BF16, kind="Internal",
                              addr_space="Shared")

    with tile.TileContext(nc) as tc, ExitStack() as ctx:
        const = ctx.enter_context(tc.tile_pool(name="const", bufs=1))
        fdtp = ctx.enter_context(tc.tile_pool(name="fdtp", bufs=1))
        sb = ctx.enter_context(tc.tile_pool(name="sb", bufs=3))
        gat = ctx.enter_context(tc.tile_pool(name="gat", bufs=3))
        epi = ctx.enter_context(tc.tile_pool(name="epi", bufs=2))
        psT = ctx.enter_context(tc.tile_pool(name="psT", bufs=2, space="PSUM"))
        psM = ctx.enter_context(tc.tile_pool(name="psM", bufs=2, space="PSUM"))
        psP = ctx.enter_context(tc.tile_pool(name="psP", bufs=2, space="PSUM"))

        # ---- stage 0: constants
        iota_i = const.tile([128, 128], I16)
        nc.gpsimd.iota(iota_i[:], pattern=[[1, 128]], base=0, channel_multiplier=0)
        iota_b = const.tile([128, 128], BF16)
        nc.vector.tensor_copy(iota_b[:], iota_i[:])

        pc_i = const.tile([128, 1], I16)
        nc.gpsimd.iota(pc_i[:], pattern=[[0, 1]], base=0, channel_multiplier=1)
        pc_f = const.tile([128, 1], F32)
        nc.vector.tensor_copy(pc_f[:], pc_i[:])
        ident = const.tile([128, 128], BF16)
        nc.vector.tensor_scalar(ident[:], iota_b[:], pc_f[:], None, op0=ALU.is_equal)

        b_sb = const.tile([1, D], F32)
        nc.sync.dma_start(b_sb[:], bvec_d[:])
        a_sb = const.tile([1, 1], F32)
        nc.sync.dma_start(a_sb[:], a_d[:])
        b_bc = const.tile([128, D], F32)
        nc.gpsimd.partition_broadcast(b_bc[:], b_sb[:])
        a_col = const.tile([128, 1], F32)
        nc.gpsimd.partition_broadcast(a_col[:], a_sb[:])

        # W: fp32 tiles -> bf16 halves, and [W | 0] widened variants
        w_f = [const.tile([128, D], F32, tag=f"wf{q}", name=f"wf{q}") for q in range(2)]
        w_b = [const.tile([128, D], BF16, tag=f"wb{q}", name=f"wb{q}") for q in range(2)]
        w_z = [const.tile([128, D2], BF16, tag=f"wz{q}", name=f"wz{q}") for q in range(2)]
        for q in range(2):
            nc.sync.dma_start(w_f[q][:], w_d[q * 128:(q + 1) * 128, :])
            nc.vector.tensor_copy(w_b[q][:], w_f[q][:])
            nc.vector.memset(w_z[q][:], 0.0)
            nc.vector.tensor_copy(w_z[q][:, 0:D], w_f[q][:])

        idx_t = {}
        off_t = {}
        inv_t = {}
        for g, (idx_d_, off_d_, inv_d_, K) in (("p", (idxp_d, offp_d, invp_d, K_p)),
                                               ("n", (idxn_d, offn_d, invn_d, K_n))):
            idx_t[g] = const.tile([128, B, K * 8], I16, tag=f"idx{g}", name=f"idx{g}")
            nc.sync.dma_start(idx_t[g][:], idx_d_[:])
            off_t[g] = const.tile([128, B * K], F32, tag=f"off{g}", name=f"off{g}")
            nc.sync.dma_start(off_t[g][:], off_d_[:])
            inv_t[g] = const.tile([128, B], F32, tag=f"inv{g}", name=f"inv{g}")
            nc.sync.dma_start(inv_t[g][:], inv_d_[:])

        def transpose_128x256_to_bf16(src_f32_tile, tag):
            """fp32 [128, 256] -> two bf16 [128,128] transposed tiles."""
            cast = sb.tile([128, D], BF16, tag="cast")
            nc.vector.tensor_copy(cast[:], src_f32_tile[:])
            outs = []
            for q in range(2):
                pt = psT.tile([128, 128], BF16, tag="pt", name="pt")
                nc.tensor.transpose(pt[:], cast[:, q * 128:(q + 1) * 128], ident[:])
                if tag.startswith("fd"):
                    o = fdtp.tile([128, 128], BF16, tag=f"{tag}{q}", name=f"{tag}{q}")
                else:
                    o = sb.tile([128, 128], BF16, tag=f"tr{q}", name=f"tr{q}")
                nc.vector.tensor_copy(o[:], pt[:])
                outs.append(o)
            return outs

        # ---- stage A: build local table slice [m | nsrc]
        for t in range(SRC_PER_CORE // 128 if do_tableA else 0):
            fs = sb.tile([128, D], F32, tag="fs")
            nc.sync.dma_start(fs[:], feat_src_d[t * 128:(t + 1) * 128, :])
            fT = transpose_128x256_to_bf16(fs, "tr")
            ps_m = psM.tile([128, D], F32, tag="m")
            nc.tensor.matmul(ps_m[:], fT[0][:], w_b[0][:], start=True, stop=False)
            nc.tensor.matmul(ps_m[:], fT[1][:], w_b[1][:], start=False, stop=True)
            m_b = sb.tile([128, D], BF16, tag="mb")
            nc.scalar.activation(m_b[:], ps_m[:], AF.Copy)
            v = sb.tile([128, D], F32, tag="v")
            nc.vector.tensor_tensor(v[:], ps_m[:], b_bc[:], ALU.add)
            h = sb.tile([128, D], F32, tag="h")
            nc.vector.tensor_scalar(h[:], v[:], a_col[:], None, op0=ALU.mult)
            nc.vector.tensor_tensor(h[:], v[:], h[:], ALU.max)
            scr = sb.tile([128, D], F32, tag="scr")
            ss = sb.tile([128, 1], F32, tag="ss")
            nc.vector.tensor_tensor(scr[:], h[:], h[:], ALU.mult)
            nc.vector.tensor_reduce(ss[:], scr[:], mybir.AxisListType.X, ALU.add)
            nc.vector.tensor_scalar(ss[:], ss[:], EPS * EPS, None, op0=ALU.max)
            nrm = sb.tile([128, 1], F32, tag="nrm")
            nc.scalar.activation(nrm[:], ss[:], AF.Sqrt)
            inv = sb.tile([128, 1], F32, tag="inv")
            nc.vector.reciprocal(inv[:], nrm[:])
            nb = sb.tile([128, D], BF16, tag="nb")
            nc.vector.tensor_scalar(nb[:], h[:], inv[:], None, op0=ALU.mult)
            nc.sync.dma_start(tbl_local[t * 128:(t + 1) * 128, 0:D], m_b[:])
            nc.sync.dma_start(tbl_local[t * 128:(t + 1) * 128, D:D2], nb[:])

        # ---- stage B: replicate the table
        if do_ag:
            nc.gpsimd.collective_compute(
            "AllGather", ALU.bypass, replica_groups=[list(range(C))],
                ins=[tbl_local[:]], outs=[tbl_full[:]],
            )

        # ---- stage C: transposed dst features (for the m_dst PSUM seed)
        fdT = []
        for b in range(B if do_stageC else 0):
            fd = sb.tile([128, D], F32, tag="fd")
            nc.sync.dma_start(fd[:], feat_dst_d[b])
            fdT.append(transpose_128x256_to_bf16(fd, f"fd{b}_"))

        # ---- stage D: main per-bin loop
        pos_all = const.tile([128, B], F32)
        neg1_all = const.tile([128, B], F32)
        neg2_all = const.tile([128, B], F32)
        for t_ in (pos_all, neg1_all, neg2_all):
            nc.vector.memset(t_[:], 0.0)

        for b in range(n_bins if do_main else 0):
            ps = {}
            for g, K in (("p", K_p), ("n", K_n)):
                gt = gat.tile([128, K, D2], BF16, tag="g")
                nc.gpsimd.dma_gather(
                    out_ap=gt[:], in_ap=tbl_full[:], idxs_ap=idx_t[g][:, b, :],
                    num_idxs=K * 128, num_idxs_reg=K * 128, elem_size=D2,
                    single_packet=False,
                )
                if d_level < 2:
                    continue
                acc = psP.tile([128, D2], F32, tag=f"ps{g}", name=f"ps{g}")
                nc.tensor.matmul(acc[:], fdT[b][0][:], w_z[0][:], start=True, stop=False)
                nc.tensor.matmul(acc[:], fdT[b][1][:], w_z[1][:], start=False, stop=False)
                for k in range(K):
                    p = sb.tile([128, 128], BF16, tag="p")
                    nc.vector.tensor_scalar(p[:], iota_b[:],
                                            off_t[g][:, b * K + k:b * K + k + 1],
                                            None, op0=ALU.is_equal)
                    nc.tensor.matmul(acc[:], p[:], gt[:, k, :],
                                     start=False, stop=(k == K - 1))
                ps[g] = acc
            if d_level < 3:
                continue

            hn = {}
            for g in ("p", "n"):
                t1 = epi.tile([128, D], F32, tag=f"t1{g}", name=f"t1{g}")
                nc.scalar.activation(t1[:], ps[g][:, 0:D], AF.Copy,
                                     scale=inv_t[g][:, b:b + 1])
                v = epi.tile([128, D], F32, tag=f"v{g}", name=f"v{g}")
                nc.vector.tensor_tensor(v[:], t1[:], b_bc[:], ALU.add)
                h = epi.tile([128, D], F32, tag=f"h{g}", name=f"h{g}")
                nc.vector.tensor_scalar(h[:], v[:], a_col[:], None, op0=ALU.mult)
                nc.vector.tensor_tensor(h[:], v[:], h[:], ALU.max)
                if g == "p":
                    nc.sync.dma_start(outh_d[b], h[:])
                if d_level < 4:
                    continue
                scr = epi.tile([128, D], F32, tag=f"scr{g}", name=f"scr{g}")
                ss = epi.tile([128, 1], F32, tag=f"ss{g}", name=f"ss{g}")
                nc.vector.tensor_tensor(scr[:], h[:], h[:], ALU.mult)
                nc.vector.tensor_reduce(ss[:], scr[:], mybir.AxisListType.X, ALU.add)
                nc.vector.tensor_scalar(ss[:], ss[:], EPS * EPS, None, op0=ALU.max)
                nrm = epi.tile([128, 1], F32, tag=f"nrm{g}", name=f"nrm{g}")
                nc.scalar.activation(nrm[:], ss[:], AF.Sqrt)
                inv = epi.tile([128, 1], F32, tag=f"invc{g}", name=f"invc{g}")
                nc.vector.reciprocal(inv[:], nrm[:])
                hng = epi.tile([128, D], F32, tag=f"hn{g}", name=f"hn{g}")
                nc.vector.tensor_scalar(hng[:], h[:], inv[:], None, op0=ALU.mult)
                hn[g] = hng
            if d_level < 5:
                continue

            scr0 = epi.tile([128, D], F32, tag="scr0")
            nc.vector.tensor_tensor(scr0[:], hn["p"][:], hn["n"][:], ALU.mult)
            nc.vector.tensor_reduce(pos_all[:, b:b + 1], scr0[:],
                                    mybir.AxisListType.X, ALU.add)
            scr1 = epi.tile([128, D], F32, tag="scr1")
            nc.vector.tensor_tensor(scr1[:], hn["p"][:], ps["n"][:, D:D2], ALU.mult)
            nc.vector.tensor_reduce(neg1_all[:, b:b + 1], scr1[:],
                                    mybir.AxisListType.X, ALU.add)
            scr2 = epi.tile([128, D], F32, tag="scr2")
            nc.vector.tensor_tensor(scr2[:], hn["n"][:], ps["p"][:, D:D2], ALU.mult)
            nc.vector.tensor_reduce(neg2_all[:, b:b + 1], scr2[:],
                                    mybir.AxisListType.X, ALU.add)

        # ---- stage E: partial reductions
        n1f = const.tile([128, B], F32)
        nc.vector.tensor_tensor(n1f[:], neg1_all[:], pos_all[:], ALU.add)
        n2f = const.tile([128, B], F32)
        nc.vector.tensor_tensor(n2f[:], neg2_all[:], pos_all[:], ALU.add)
        cols = const.tile([128, 3], F32)
        for i, t_ in enumerate((pos_all, n1f, n2f)):
            esc = const.tile([128, B], F32, tag=f"esc{i}", name=f"esc{i}")
            nc.scalar.activation(esc[:], t_[:], AF.Exp)
            nc.vector.tensor_reduce(cols[:, i:i + 1], esc[:],
                                    mybir.AxisListType.X, ALU.add)
        partials = const.tile([128, 2], F32)
        nc.vector.tensor_tensor(partials[:, 0:1], cols[:, 0:1], cols[:, 1:2], ALU.add)
        nc.vector.tensor_tensor(partials[:, 0:1], partials[:, 0:1], cols[:, 2:3],
                                ALU.add)
        nc.vector.tensor_reduce(partials[:, 1:2], pos_all[:],
                                mybir.AxisListType.X, ALU.add)
        nc.sync.dma_start(outp_d[:], partials[:])

    nc.compile()
    return nc


_CACHE = {}


def _make_inmaps(feat, W, b, a, plan):
    K_p, K_n = plan["K_p"], plan["K_n"]
    idx_p = _wrap_idx(plan["idx_p"])            # [C, 128, B, K_p*8]
    idx_n = _wrap_idx(plan["idx_n"])
    off_p = _layout_off(plan["off_p"], K_p)     # [C, 128, B*K_p]
    off_n = _layout_off(plan["off_n"], K_n)
    inv_p = plan["invd_p"].reshape(C, B, S).transpose(0, 2, 1)  # [C, 128, B]
    inv_n = plan["invd_n"].reshape(C, B, S).transpose(0, 2, 1)
    nodes_at = plan["nodes_at"]

    in_maps = []
    for c in range(C):
        bins = nodes_at[c * B:(c + 1) * B]      # [B, S]
        in_maps.append(dict(
            feat_src=feat[c * SRC_PER_CORE:(c + 1) * SRC_PER_CORE],
            feat_dst=np.ascontiguousarray(feat[N_SRC + bins]),
            w=W, bvec=b, a=a,
            idx_p=np.ascontiguousarray(idx_p[c]),
            idx_n=np.ascontiguousarray(idx_n[c]),
            off_p=np.ascontiguousarray(off_p[c]),
            off_n=np.ascontiguousarray(off_n[c]),
            invd_p=np.ascontiguousarray(inv_p[c]),
            invd_n=np.ascontiguousarray(inv_n[c]),
        ))
    return in_maps


def _assemble(plan, per_core_outs):
    nodes_at = plan["nodes_at"]
    h_full = np.empty((N_DST, D), np.float32)
    s_exp = 0.0
    s_pos = 0.0
    for c in range(C):
        out = per_core_outs[c]
        bins = nodes_at[c * B:(c + 1) * B]
        h_full[bins.reshape(-1)] = np.asarray(out["out_h"]).reshape(-1, D)
        part = np.asarray(out["out_partials"])
        s_exp += float(part[:, 0].sum(dtype=np.float64))
        s_pos += float(part[:, 1].sum(dtype=np.float64))
    loss = np.float32(np.log(s_exp) - s_pos)
    return loss, h_full


def kernel(feat, src, dst, neg_src, neg_dst, W, b, prelu_a, _trace=False):
    global LAST_RESULTS
    feat = np.ascontiguousarray(np.asarray(feat, np.float32))
    W = np.ascontiguousarray(np.asarray(W, np.float32))
    b = np.asarray(b, np.float32).reshape(1, D)
    a = np.asarray(prelu_a, np.float32).reshape(1, 1)

    plan = _preprocess(np.asarray(src), np.asarray(dst), np.asarray(neg_src),
                       np.asarray(neg_dst))
    K_p, K_n = plan["K_p"], plan["K_n"]
    if (K_p, K_n) not in _CACHE:
        _CACHE[(K_p, K_n)] = _build(K_p, K_n)
    nc = _CACHE[(K_p, K_n)]

    in_maps = _make_inmaps(feat, W, b, a, plan)
    res = run_bass_kernel_spmd(nc, in_maps, core_ids=list(range(C)), trace=_trace)
    LAST_RESULTS = res
    return _assemble(plan, res.results)
